# revision 29
# baseline (speedup 1.0000x reference)
"""Trainium2 Bass kernel for imagen-style self-attention with pos_bias.

Reference computation (fp32 jax):
    xn   = LN(x) * g_norm                      # gamma-only layernorm
    qkv  = xn @ w_qkv ; q,k,v per head (h=8, d=64) ; q *= d**-0.5
    sim  = q @ k^T + pos_bias[h]               # [b, h, n, n]
    attn = softmax(sim, -1)
    out  = LN((attn @ v) @ w_out) * g_out

Distribution: 8 cores, one head per core (tensor parallel over heads).
Each core computes LN+QKV projection for its head over the full batch,
full attention for its head, then AllToAlls (per 512-row i-span) re-shard
by sequence rows; each core runs the output projection + final LN for a
1/8 row shard.

Perf structure (v2):
 - q/k stored in partition halves (batch pairs) so the K=64 QK^T matmuls
   run two batches concurrently via PE row tiling (tile_position).
 - PV matmuls are M=64 col-tiled pairs (two batches per psum bank);
   softmax denominators come from 4x col-tiled M=1 ones-matmuls.
 - exp() runs 1024 wide across two adjacent psum banks.
 - O is sent UNNORMALIZED through the AllToAll together with the
   denominator row; normalization happens in phase 4 (final LN is
   invariant per-head-row scale folds there), removing the DRAM
   round-trip reciprocal chain from the phase-2 critical path.

Row shard mapping: core c owns rows (b=c//2, i in [512*ii + 256*(c%2),
+256)) for all ii, in ii order.
"""

import numpy as np

import concourse.bass as bass
import concourse.bacc as bacc
import concourse.mybir as mybir
import concourse.tile as tile
from concourse.bass_utils import run_bass_kernel_spmd
from concourse.masks import make_identity

B = 4
N = 2048
D = 512
HEADS = 8
DH = 64
SCALE = DH**-0.5
EPS = 1e-5
NCORES = 8

F16 = mybir.dt.float16
F32 = mybir.dt.float32
AF = mybir.ActivationFunctionType
ALU = mybir.AluOpType


def _phase1(nc, tc, x_d, w_sb, eps_t, ident, qT_sb, kT_sb, v_sb, n, b,
            mid_hook=None):
    """LN + transpose + QKV projection over all rows.

    qT_sb/kT_sb layout: [128, 2, n] fp16 where partitions 0:64 hold even
    batches (0,2) and 64:128 hold odd batches (1,3); free dim 0 indexes
    the batch pair g=bi//2.
    """
    rows = b * n
    n_spans = rows // 512
    with (
        tc.tile_pool(name="p1", bufs=3) as p1,
        tc.tile_pool(name="p1xT", bufs=3) as p1xT,
        tc.tile_pool(name="ps_t", bufs=4, space="PSUM") as ps_t,
        tc.tile_pool(name="ps_p", bufs=2, space="PSUM") as ps_p,
    ):
        for sp in range(n_spans):
            if sp == 8 and mid_hook is not None:
                mid_hook()
            bi = (sp * 512) // n
            cols = slice((sp * 512) % n, (sp * 512) % n + 512)
            rg = (bi % 2) * 64  # partition half for this batch
            g = bi // 2  # batch pair

            xnT = p1xT.tile([128, 4, 512], F16, tag="xnT")
            mvs = p1.tile([128, 4, 2], F32, tag="mvs")
            rstds = p1.tile([128, 4], F32, tag="rstds")
            nmr = p1.tile([128, 4], F32, tag="nmr")
            x_ts = []
            for t in range(4):
                rt = sp * 4 + t
                x_t = p1.tile([128, D], F16, tag=f"x{t}", name=f"x{t}", bufs=4)
                x_ts.append(x_t)
                nc.sync.dma_start(out=x_t, in_=x_d[rt * 128 : (rt + 1) * 128, :])
                stats = p1.tile([128, 6], F32, tag="stats", bufs=8)
                nc.vector.bn_stats(out=stats, in_=x_t)
                nc.vector.bn_aggr(out=mvs[:, t, :], in_=stats)
            # batched rstd for the whole span: 1/sqrt(var + eps)
            nc.scalar.activation(
                out=rstds, in_=mvs[:, :, 1], func=AF.Sqrt, bias=eps_t
            )
            nc.vector.reciprocal(out=rstds, in_=rstds)
            # bias tile for ACT-side LN apply: -mean * rstd
            nc.vector.scalar_tensor_tensor(
                out=nmr,
                in0=mvs[:, :, 0],
                scalar=-1.0,
                in1=rstds,
                op0=ALU.mult,
                op1=ALU.mult,
            )
            for t in range(4):
                xn_t = p1.tile([128, D], F16, tag="xn", bufs=8)
                if t % 2 == 0:
                    nc.vector.tensor_scalar(
                        out=xn_t,
                        in0=x_ts[t],
                        scalar1=mvs[:, t, 0:1],
                        scalar2=rstds[:, t : t + 1],
                        op0=ALU.subtract,
                        op1=ALU.mult,
                    )
                else:
                    # same LN apply on the scalar engine: x*rstd + (-mean*rstd)
                    nc.scalar.activation(
                        out=xn_t,
                        in_=x_ts[t],
                        func=AF.Identity,
                        bias=nmr[:, t : t + 1],
                        scale=rstds[:, t : t + 1],
                    )
                # transpose 4 chunks into one psum bank, single evac cast
                ps = ps_t.tile([128, 512], F32, tag="tp")
                for c in range(4):
                    nc.tensor.matmul(
                        ps[:, c * 128 : (c + 1) * 128],
                        lhsT=xn_t[:, c * 128 : (c + 1) * 128],
                        rhs=ident,
                        start=(c == 0),
                        stop=(c == 3),
                    )
                if t % 2 == 0:
                    nc.vector.tensor_copy(
                        out=xnT[:, :, t * 128 : (t + 1) * 128],
                        in_=ps.rearrange("p (c w) -> p c w", c=4),
                    )
                else:
                    nc.scalar.copy(
                        out=xnT[:, :, t * 128 : (t + 1) * 128],
                        in_=ps.rearrange("p (c w) -> p c w", c=4),
                    )

            # QKV projection: packed as one M=128 matmul ([q|k]) plus one
            # M=64 matmul (v) per contraction chunk.
            ps_qk = ps_p.tile([128, 512], F32, tag="qk")
            ps_v = ps_p.tile([64, 512], F32, tag="v")
            for c in range(4):
                st, fin = (c == 0), (c == 3)
                nc.tensor.matmul(
                    ps_qk, lhsT=w_sb[:, c, 0:128], rhs=xnT[:, c, :], start=st, stop=fin
                )
                nc.tensor.matmul(
                    ps_v, lhsT=w_sb[:, c, 128:192], rhs=xnT[:, c, :], start=st, stop=fin
                )
            nc.vector.tensor_copy(out=qT_sb[rg : rg + 64, g, cols], in_=ps_qk[0:64, :])
            nc.scalar.copy(out=kT_sb[rg : rg + 64, g, cols], in_=ps_qk[64:128, :])
            vT_t = p1.tile([64, 512], F16, tag="vT")
            nc.scalar.copy(out=vT_t, in_=ps_v)
            # transpose vT [64, 512] into v natural [512, 64]: 4 transposes
            # into one psum bank, single strided evac
            jc0 = ((sp * 512) % n) // 128
            ps2 = ps_t.tile([128, 4, 64], F32, tag="tp", name="psv")
            for t in range(4):
                nc.tensor.matmul(
                    ps2[:, t, :],
                    lhsT=vT_t[:, t * 128 : (t + 1) * 128],
                    rhs=ident[0:64, 0:64],
                    start=(t == 0),
                    stop=(t == 3),
                )
            nc.scalar.copy(out=v_sb[:, bi, jc0 : jc0 + 4, :], in_=ps2)


def _phase2(nc, tc, pools, post_d, ident, ones_c, qT_sb, kT_sb, v_sb, a2a_ins,
            pos_tiles, n, b, ii, n_ii):
    """Attention for one i-span (512 q rows), all 4 batches.

    Per j-tile: pos-loads + row-tiled QK pairs accumulate S for 4 batches
    into two [128,1024] psum tiles; 1024-wide exp into a [128,2048] fp16 P
    tile; col-tiled PV pairs + 4x col-tiled M=1 sums matmuls lag one j.
    O/sums stay unnormalized and are DMA'd to the a2a buffer.
    """
    n_jc = n // 128
    p2, p2o, p2c, ps_s, ps_o, ps_sum = pools
    icols = slice(ii * 512, (ii + 1) * 512)

    # prefetch pos_bias columns for this ii (emitted one ii early by caller)
    pos_c = pos_tiles[ii]

    O_bank = []
    for ob in range(2):
        O_bank.append(ps_o.tile([128, 512], F32, tag="O", name=f"O{ob}"))
    sums_bank = ps_sum.tile([128, 512], F32, tag="sums")

    prev = None  # (P_tile, j)

    def _pv(P_t, j):
        for ob in range(2):  # ob=0: batches 0,1 ; ob=1: batches 2,3
            for h in range(2):
                bi = 2 * ob + h
                nc.tensor.matmul(
                    O_bank[ob][h * 64 : (h + 1) * 64, :],
                    lhsT=v_sb[:, bi, j, :],
                    rhs=P_t[:, bi * 512 : (bi + 1) * 512],
                    start=(j == 0),
                    stop=(j == n_jc - 1),
                )
        for bi in range(4):
            nc.tensor.matmul(
                sums_bank[32 * bi : 32 * bi + 1, :],
                lhsT=ones_c,
                rhs=P_t[:, bi * 512 : (bi + 1) * 512],
                start=(j == 0),
                stop=(j == n_jc - 1),
                skip_group_check=True,
                tile_position=(0, 32 * bi),
            )

    for j in range(n_jc):
        P_t = p2.tile([128, 2048], F16, tag="P")
        jsl = slice(j * 128, (j + 1) * 128)
        S_ts = [
            ps_s.tile([128, 1024], F32, tag="S", name=f"S{half}")
            for half in range(2)
        ]
        # pos loads first (ident stays loaded across all four matmuls)
        for half in range(2):
            for h in range(2):
                nc.tensor.matmul(
                    S_ts[half][:, h * 512 : (h + 1) * 512],
                    lhsT=ident,
                    rhs=pos_c[:, j, :],
                    start=True,
                    stop=False,
                )
        for half in range(2):
            S_t = S_ts[half]
            # row-tiled QK pair: batch 2*half at rows 0:64, 2*half+1 at 64:128
            for h in range(2):
                rg = h * 64
                nc.tensor.matmul(
                    S_t[:, h * 512 : (h + 1) * 512],
                    lhsT=kT_sb[rg : rg + 64, half, jsl],
                    rhs=qT_sb[rg : rg + 64, half, icols],
                    start=False,
                    stop=True,
                )
            nc.scalar.activation(
                out=P_t[:, half * 1024 : (half + 1) * 1024], in_=S_t, func=AF.Exp
            )
        if prev is not None:
            _pv(*prev)
        prev = (P_t, j)
    _pv(*prev)

    # evac O banks + sums into per-batch [65, 512] staging tiles (row 64 =
    # softmax denominator), then one DMA per a2a destination
    for bi in range(4):
        ob, hh = bi // 2, bi % 2
        stg = p2o.tile([DH + 1, 512], F16, tag=f"stg{bi}", name=f"stg{bi}")
        nc.vector.tensor_copy(
            out=stg[0:DH, :], in_=O_bank[ob][hh * 64 : (hh + 1) * 64, :]
        )
        nc.vector.tensor_copy(
            out=stg[DH : DH + 1, :], in_=sums_bank[32 * bi : 32 * bi + 1, :]
        )
        for h in range(2):
            dest = 2 * bi + h
            nc.sync.dma_start(
                out=a2a_ins[ii][dest, :, :],
                in_=stg[:, h * 256 : (h + 1) * 256],
            )


DEBUG_T = {}


def _phase4_proj(nc, tc, pools, src, recd, sumsd, wout_sb, g_bc, eps_t, out_d, ii):
    """Out projection + final LN for one 256-row a2a chunk.

    src: a2a_out dram tensor [8, DH+1, 256] (unnormalized O + sums row).
    Normalize here: hT *= broadcast(1/sums), then project + final LN.
    """
    p4, p4h, ps_y = pools
    # read each head's whole [65, 256] chunk in ONE DMA (O rows + sums row
    # arrive together), then reshape on-chip
    hT65 = p4h.tile([DH + 1, 8, 256], F16, tag="hT65")
    for h in range(NCORES):
        nc.gpsimd.dma_start(out=hT65[:, h, :], in_=src[h, :, :])
    hT_sb = p4h.tile([128, 4, 256], F16, tag="hT")
    for two in range(2):
        for c in range(4):
            nc.vector.tensor_copy(
                out=hT_sb[two * 64 : (two + 1) * 64, c, :],
                in_=hT65[0:DH, 2 * c + two, :],
            )
    # sums row (partition DH) -> tiny DRAM row -> [64, 32] spread so the
    # iterative reciprocal stays short
    nc.gpsimd.dma_start(
        out=sumsd[ii, :].rearrange("(a b) -> a b", a=1),
        in_=hT65[DH : DH + 1, :, :].rearrange("p c r -> p (c r)"),
    )
    sumsb = p4.tile([64, 32], F16, tag="sumsb")
    nc.gpsimd.dma_start(
        out=sumsb, in_=sumsd[ii, :].rearrange("(q f) -> q f", q=64)
    )
    sums32 = p4.tile([64, 32], F32, tag="sums32")
    nc.vector.tensor_copy(out=sums32, in_=sumsb)
    rec = p4.tile([64, 32], F32, tag="rec")
    nc.vector.reciprocal(out=rec, in_=sums32)
    rec16 = p4.tile([64, 32], F16, tag="rec16")
    nc.vector.tensor_copy(out=rec16, in_=rec)
    for h in range(NCORES):
        nc.gpsimd.dma_start(
            out=recd[ii * NCORES + h, :].rearrange("(p f) -> p f", p=8),
            in_=rec16[h * 8 : (h + 1) * 8, :],
        )
    rbc = p4h.tile([128, 4, 256], F16, tag="rbc")
    for two in range(2):
        for c in range(4):
            h = 2 * c + two
            nc.gpsimd.dma_start(
                out=rbc[two * 64 : (two + 1) * 64, c, :],
                in_=recd[ii * NCORES + h, :].partition_broadcast(64),
            )
    hT_n = p4h.tile([128, 4, 256], F16, tag="hTn")
    nc.vector.tensor_tensor(out=hT_n, in0=hT_sb, in1=rbc, op=ALU.mult)
    if ii == 0 and "sumsb_o" in DEBUG_T:
        nc.gpsimd.dma_start(out=DEBUG_T["sumsb_o"][:, :], in_=sumsb)
        nc.gpsimd.dma_start(out=DEBUG_T["rec16_o"][:, :], in_=rec16)
        recchk = p4.tile([64, 32], F16, tag="recchk")
        nc.gpsimd.dma_start(
            out=recchk.rearrange("(h p) f -> h p f", h=8),
            in_=recd[ii * NCORES : ii * NCORES + 8, :].rearrange(
                "h (p f) -> h p f", f=32
            ),
        )
        nc.gpsimd.dma_start(out=DEBUG_T["recd_o"][:, :], in_=recchk)
    if ii == 0 and "hTn_o" in DEBUG_T:
        nc.gpsimd.dma_start(
            out=DEBUG_T["hTn_o"][:, :], in_=hT_n.rearrange("p a b -> p (a b)")
        )
        nc.gpsimd.dma_start(
            out=DEBUG_T["hT_o"][:, :], in_=hT_sb.rearrange("p a b -> p (a b)")
        )
        nc.gpsimd.dma_start(
            out=DEBUG_T["rbc_o"][:, :], in_=rbc.rearrange("p a b -> p (a b)")
        )

    for it in range(2):
        ps = ps_y.tile([128, D], F32, tag="y")
        for c in range(4):
            nc.tensor.matmul(
                ps,
                lhsT=hT_n[:, c, it * 128 : (it + 1) * 128],
                rhs=wout_sb[:, c, :],
                start=(c == 0),
                stop=(c == 3),
            )
        stats = p4.tile([128, 6], F32, tag="stats4")
        nc.vector.bn_stats(out=stats, in_=ps)
        mv = p4.tile([128, 2], F32, tag="mv4")
        nc.vector.bn_aggr(out=mv, in_=stats)
        rstd = p4.tile([128, 1], F32, tag="rstd4")
        nc.scalar.activation(out=rstd, in_=mv[:, 1:2], func=AF.Sqrt, bias=eps_t)
        nc.vector.reciprocal(out=rstd, in_=rstd)
        y_t = p4.tile([128, D], F32, tag="y4")
        if ii == 0 and "y_o" in DEBUG_T:
            yd = p4.tile([128, D], F32, tag="yd", name=f"yd{it}")
            nc.vector.tensor_copy(out=yd, in_=ps)
            nc.gpsimd.dma_start(
                out=DEBUG_T["y_o"][it * 128 : (it + 1) * 128, :], in_=yd
            )
        nc.vector.tensor_scalar(
            out=y_t,
            in0=ps,
            scalar1=mv[:, 0:1],
            scalar2=rstd,
            op0=ALU.subtract,
            op1=ALU.mult,
        )
        nc.vector.tensor_tensor(out=y_t, in0=y_t, in1=g_bc, op=ALU.mult)
        row0 = ii * 256 + it * 128
        nc.gpsimd.dma_start(out=out_d[row0 : row0 + 128, :], in_=y_t)


def build_attention_bass(n: int = N, b: int = B) -> bass.Bass:
    """Build the SPMD per-core Bass program (identical on all cores)."""
    rows = b * n
    assert rows % (NCORES * 128) == 0 and n % 512 == 0 and b == 4
    rows_pc = rows // NCORES
    n_ii = n // 512
    n_jc = n // 128

    nc = bacc.Bacc(num_devices=NCORES)

    x_d = nc.declare_dram_parameter("x", [rows, D], F16, isOutput=False)
    w_d = nc.declare_dram_parameter("w", [4, 128, 3 * DH], F16, isOutput=False)
    post_d = nc.declare_dram_parameter("post", [n, n], F16, isOutput=False)
    wout_d = nc.declare_dram_parameter("wout", [4, 128, D], F16, isOutput=False)
    g_d = nc.declare_dram_parameter("g", [1, D], F32, isOutput=False)
    out_d = nc.declare_dram_parameter("out", [rows_pc, D], F32, isOutput=True)
    if _DEBUG_BUILD:
        DEBUG_T["ssb_o"] = nc.declare_dram_parameter("ssb_o", [128, 512], F16, isOutput=True)
        DEBUG_T["a2ai_chk"] = nc.declare_dram_parameter("a2ai_chk", [128, 256], F16, isOutput=True)
        DEBUG_T["hTn_o"] = nc.declare_dram_parameter("hTn_o", [128, 1024], F16, isOutput=True)
        DEBUG_T["hT_o"] = nc.declare_dram_parameter("hT_o", [128, 1024], F16, isOutput=True)
        DEBUG_T["rbc_o"] = nc.declare_dram_parameter("rbc_o", [128, 1024], F16, isOutput=True)
        DEBUG_T["y_o"] = nc.declare_dram_parameter("y_o", [256, D], F32, isOutput=True)
        DEBUG_T["sumsb_o"] = nc.declare_dram_parameter("sumsb_o", [64, 32], F16, isOutput=True)
        DEBUG_T["rec16_o"] = nc.declare_dram_parameter("rec16_o", [64, 32], F16, isOutput=True)
        DEBUG_T["recd_o"] = nc.declare_dram_parameter("recd_o", [64, 32], F16, isOutput=True)

    a2a_ins = [
        nc.dram_tensor(f"a2a_in{ii}", [NCORES, DH + 1, 256], F16)
        for ii in range(n_ii)
    ]
    a2a_outs = [
        nc.dram_tensor(f"a2a_out{ii}", [NCORES, DH + 1, 256], F16)
        for ii in range(n_ii)
    ]
    recd = nc.dram_tensor("recd", [n_ii * NCORES, 256], F16)
    sumsd = nc.dram_tensor("sumsd", [n_ii, 8 * 256], F16)

    with tile.TileContext(nc) as tc:
        with (
            tc.tile_pool(name="singles", bufs=1) as singles,
            tc.tile_pool(name="persist", bufs=1) as persist,
        ):
            ident = singles.tile([128, 128], F16)
            make_identity(nc, ident)
            eps_t = singles.tile([128, 1], F32)
            nc.vector.memset(eps_t, EPS)
            ones_c = singles.tile([128, 1], F16)
            nc.vector.memset(ones_c, 1.0)
            w_sb = singles.tile([128, 4, 3 * DH], F16)
            nc.sync.dma_start(out=w_sb, in_=w_d.rearrange("c p m -> p c m"))

            qT_sb = persist.tile([128, 2, n], F16, name="qT")
            kT_sb = persist.tile([128, 2, n], F16, name="kT")
            v_sb = persist.tile([128, b, n_jc, DH], F16, name="v")

            with (
                tc.tile_pool(name="p2c", bufs=2) as p2c,
                tc.tile_pool(name="p4s", bufs=1) as p4s,
            ):
                # pos_bias column-block prefetch, one ii ahead
                pos_tiles = {}

                def _prefetch_pos(ii):
                    if ii >= n_ii or ii in pos_tiles:
                        return
                    pt = p2c.tile([128, n_jc, 512], F16, tag="posc",
                                  name=f"posc{ii}")
                    icols = slice(ii * 512, (ii + 1) * 512)
                    for j in range(n_jc):
                        nc.sync.dma_start(
                            out=pt[:, j, :],
                            in_=post_d[j * 128 : (j + 1) * 128, icols],
                        )
                    pos_tiles[ii] = pt

                wout_sb = p4s.tile([128, 4, D], F16)
                nc.sync.dma_start(
                    out=wout_sb, in_=wout_d.rearrange("c p m -> p c m")
                )
                g_bc = p4s.tile([128, D], F32)
                nc.sync.dma_start(
                    out=g_bc, in_=g_d[0, :].partition_broadcast(128)
                )
                _phase1(nc, tc, x_d, w_sb, eps_t, ident, qT_sb, kT_sb, v_sb,
                        n, b)

                with (
                    tc.tile_pool(name="p2", bufs=2) as p2,
                    tc.tile_pool(name="p2o", bufs=2) as p2o,
                    tc.tile_pool(name="ps_s", bufs=2, space="PSUM") as ps_s,
                    tc.tile_pool(name="ps_o", bufs=2, space="PSUM") as ps_o,
                    tc.tile_pool(name="ps_sum", bufs=1, space="PSUM") as ps_sum,
                    tc.tile_pool(name="p4", bufs=3) as p4,
                    tc.tile_pool(name="p4h", bufs=2) as p4h,
                    tc.tile_pool(name="ps_y", bufs=1, space="PSUM") as ps_y,
                ):
                    p2pools = (p2, p2o, p2c, ps_s, ps_o, ps_sum)
                    p4pools = (p4, p4h, ps_y)

                    DEBUG_T["_a2a_in0"] = a2a_ins[0]
                    P4LAG = 2
                    _prefetch_pos(0)
                    for ii in range(n_ii):
                        _prefetch_pos(ii + 1)
                        _phase2(
                            nc, tc, p2pools, post_d, ident, ones_c, qT_sb, kT_sb,
                            v_sb, a2a_ins, pos_tiles, n, b, ii, n_ii,
                        )
                        pos_tiles.pop(ii, None)
                        nc.gpsimd.collective_compute(
                            "AllToAll",
                            ALU.bypass,
                            replica_groups=[list(range(NCORES))],
                            ins=[a2a_ins[ii][:]],
                            outs=[a2a_outs[ii][:]],
                        )
                        if ii >= P4LAG:
                            _phase4_proj(
                                nc, tc, p4pools, a2a_outs[ii - P4LAG], recd,
                                sumsd, wout_sb, g_bc, eps_t, out_d, ii - P4LAG,
                            )
                    for ii in range(max(0, n_ii - P4LAG), n_ii):
                        _phase4_proj(
                            nc, tc, p4pools, a2a_outs[ii], recd, sumsd, wout_sb,
                            g_bc, eps_t, out_d, ii,
                        )
                    if _DEBUG_BUILD:
                        a2achk = p4.tile([128, 256], F16, tag="a2achk")
                        nc.sync.dma_start(
                            out=a2achk.rearrange("(a b) r -> a b r", a=8),
                            in_=a2a_ins[0][:, 49:65, :],
                        )
                        nc.sync.dma_start(out=DEBUG_T["a2ai_chk"][:, :], in_=a2achk)

    nc.finalize()
    return nc


def make_in_maps(x, pos_bias, w_qkv, w_out, g_norm, g_out, n=N, b=B):
    """Host-side shard/layout prep: per-core input maps (no math beyond
    folding the LN gamma / attention scale diagonals into the weights)."""
    rows = b * n
    x16 = np.ascontiguousarray(x.reshape(rows, D)).astype(np.float16)
    w_eff = w_qkv * g_norm[:, None].astype(np.float32)
    wout16 = np.ascontiguousarray(w_out.reshape(4, 128, D)).astype(np.float16)
    g_row = np.ascontiguousarray(g_out.reshape(1, D)).astype(np.float32)
    hidden = HEADS * DH
    in_maps = []
    for h in range(NCORES):
        wq = w_eff[:, h * DH : (h + 1) * DH] * SCALE
        wk = w_eff[:, hidden + h * DH : hidden + (h + 1) * DH]
        wv = w_eff[:, 2 * hidden + h * DH : 2 * hidden + (h + 1) * DH]
        w_h = np.concatenate([wq, wk, wv], axis=1).reshape(4, 128, 3 * DH)
        posT = np.ascontiguousarray(pos_bias[h].T).astype(np.float16)
        in_maps.append(
            {
                "x": x16,
                "w": np.ascontiguousarray(w_h).astype(np.float16),
                "post": posT,
                "wout": wout16,
                "g": g_row,
            }
        )
    return in_maps


def assemble_output(results, n=N, b=B):
    """Scatter per-core row shards back to the full [b, n, D] output."""
    out = np.empty((b, n, D), dtype=np.float32)
    n_ii = n // 512
    for c in range(NCORES):
        oc = results[c]["out"]
        bi = c // 2
        for ii in range(n_ii):
            i0 = 512 * ii + 256 * (c % 2)
            out[bi, i0 : i0 + 256, :] = oc[ii * 256 : (ii + 1) * 256, :]
    return out


_DEBUG_BUILD = False
_NC_CACHE: dict = {}


def _get_nc(n=N, b=B):
    key = (n, b)
    if key not in _NC_CACHE:
        _NC_CACHE[key] = build_attention_bass(n, b)
    return _NC_CACHE[key]


def kernel(x, pos_bias, w_qkv, w_out, g_norm, g_out, _trace=False):
    x = np.asarray(x, dtype=np.float32)
    pos_bias = np.asarray(pos_bias, dtype=np.float32)
    w_qkv = np.asarray(w_qkv, dtype=np.float32)
    w_out = np.asarray(w_out, dtype=np.float32)
    g_norm = np.asarray(g_norm, dtype=np.float32)
    g_out = np.asarray(g_out, dtype=np.float32)
    b, n, _ = x.shape

    nc = _get_nc(n, b)
    in_maps = make_in_maps(x, pos_bias, w_qkv, w_out, g_norm, g_out, n, b)
    res = run_bass_kernel_spmd(
        nc, in_maps, core_ids=list(range(NCORES)), trace=_trace
    )
    if _trace:
        kernel.last_results = res
    return assemble_output(res.results, n, b)
